# revision 37
# baseline (speedup 1.0000x reference)
"""BatchBlur: depthwise 15x15 conv with per-sample kernels, reflection pad 7.

x: (32, 3, 512, 512) f32, kernel: (32, 15, 15) f32 -> out (32, 3, 512, 512) f32.

Strategy: pure data parallel over batch, 4 samples per core on 8 cores.
Host: reflection-pad x to (., 526, 526), cast to fp16, and build dual-band
matrices A[s, k, j, m]: for k<64, A = kern[s, k-m, 2j]; for k>=64,
A = kern[s, k-64-m, 2j+1] (band condition 0 <= dy < 15).
Device: each rhs tile holds the strip rows TWICE — partitions 0:64 at
column offset 0 and partitions 64:128 at column offset 1 (two DMAs straight
from DRAM) — so a single accumulating matmul covers TWO horizontal taps
(dx=2j in the lower band, dx=2j+1 in the upper band) for 50 output rows:
  out[m, n] += sum_k A[k, j, m] * rhs[k, n + 2j]
Eight streams (j=0..7; j=7 contracts only the lower 64 partitions) replace
the fifteen per-tap matmuls. Two images run concurrently in the two
64-column halves of the PE array via tile_position=(0,0)/(0,64). fp16
operands keep the PE at 1 cycle/row (fp32 PSUM accumulation, ~3e-4 relative
error). Strips: output rows 0..499 in ten 50-row strips plus one final
strip at r0=462 (rows 462..525 = the exact end of the padded image) whose
store is sliced to rows 500..511.
"""
import os
import sys

for _p in ("/opt/trn_rl_repo", "/root/.axon_site/_ro/trn_rl_repo"):
    if _p not in sys.path and os.path.isdir(_p):
        sys.path.insert(0, _p)

import numpy as np

import concourse.bass as bass
import concourse.mybir as mybir
import concourse.tile as tile
from concourse import bacc
from concourse.bass_utils import run_bass_kernel_spmd

L = 15           # blur kernel size
P = L // 2       # reflection pad
B, C, H, W = 32, 3, 512, 512
N_CORES = 8
BS = B // N_CORES            # samples per core
NIMG = BS * C                # channel images per core
HP, WP = H + 2 * P, W + 2 * P  # 526
M_STRIP = 50                 # output rows per strip (dual-band: 2*(50+14)=128)
K_GRP = M_STRIP + L - 1      # 64 input rows per band group
N_DX = (L + 1) // 2          # 8 streams (two taps each; last is single)
R0S = [50 * s for s in range(10)] + [HP - K_GRP]  # last strip 462..525 exactly
N_WARMUP = 100               # dummy matmuls to release the HAM clock gate

F16 = mybir.dt.float16
F32 = mybir.dt.float32

_program_cache = None


def _build_program():
    nc = bacc.Bacc("TRN2", target_bir_lowering=False, debug=False)
    xp_d = nc.dram_tensor("xp", [NIMG, HP, WP], F16, kind="ExternalInput").ap()
    a_d = nc.dram_tensor("a", [BS, 128, N_DX, M_STRIP], F16,
                         kind="ExternalInput").ap()
    out_d = nc.dram_tensor("out", [NIMG, H, W], F32, kind="ExternalOutput").ap()

    def load_strip(t, img, r0):
        # lower band: rows at column offset 0 (Sync queue); upper band: same
        # rows at column offset 1 (GpSimd queue) => one matmul covers two
        # horizontal taps. Separate queues keep issue bandwidth in reserve.
        nc.sync.dma_start(out=t[0:K_GRP, :], in_=xp_d[img, r0:r0 + K_GRP, :])
        nc.gpsimd.dma_start(out=t[K_GRP:2 * K_GRP, 0:WP - 1],
                            in_=xp_d[img, r0:r0 + K_GRP, 1:WP])

    def load_strip2(t, img, r0):
        # double-strip load: one DMA per band group brings rows for strips
        # r0 and r0+50 (free-dim blocks 0:WP and WP:2*WP). The DRAM source
        # is an overlapping strided view (row stride WP, strip stride 50*WP)
        # — plain byte streams, legal for reads. Halves the DMA issue rate.
        base = (img * HP + r0) * WP
        nc.sync.dma_start(
            out=t[0:K_GRP, :].rearrange("p (q c) -> p q c", c=WP),
            in_=bass.AP(xp_d.tensor, base,
                        [[WP, K_GRP], [50 * WP, 2], [1, WP]]))
        nc.gpsimd.dma_start(
            out=t[K_GRP:2 * K_GRP, :].rearrange(
                "p (q c) -> p q c", c=WP)[:, :, 0:WP - 1],
            in_=bass.AP(xp_d.tensor, base + 1,
                        [[WP, K_GRP], [50 * WP, 2], [1, WP - 1]]))

    with tile.TileContext(nc) as tc:
        with (
            tc.tile_pool(name="aconst", bufs=1) as apool,
            tc.tile_pool(name="warm", bufs=1) as wpool,
            tc.tile_pool(name="xin", bufs=8) as xpool,
            tc.tile_pool(name="oout", bufs=4) as opool,
            tc.tile_pool(name="psum", bufs=6, space="PSUM") as psum,
            tc.tile_pool(name="psumw", bufs=1, space="PSUM") as psumw,
        ):
            # HAM warm-up: a burst of full-array matmuls on a zeroed scratch
            # tile releases the PE clock gate (col-tiled matmuls are invisible
            # to the HAM) while the first input DMAs are in flight.
            wsrc = wpool.tile([128, 64], mybir.dt.bfloat16)
            nc.gpsimd.memset(wsrc[:], 0.0)
            wacc = psumw.tile([64, 64], F32)
            for _ in range(N_WARMUP):
                nc.tensor.matmul(wacc[:], wsrc[:, :64], wsrc[:], start=True,
                                 stop=True)

            # The upper-band DMAs write columns 0..524 of each strip block
            # only; the last column of each block is read (x 0.0 weight) by
            # the j=7 stream, so it must be finite. Zero it once per slot.
            for slot in range(8):
                t = xpool.tile([128, 2 * WP], F16, tag="xp2", name="xz2")
                nc.gpsimd.memset(t[K_GRP:2 * K_GRP, WP - 1:WP], 0.0)
                nc.gpsimd.memset(t[K_GRP:2 * K_GRP, 2 * WP - 1:2 * WP], 0.0)
            for slot in range(8):
                t = xpool.tile([128, WP], F16, tag="xp_t", name="xz1")
                nc.gpsimd.memset(t[K_GRP:2 * K_GRP, WP - 1:WP], 0.0)

            # first double-strip's image rows: issued before the A load so
            # the DMA queues deliver the first matmuls' dependencies earliest
            xp_first = []
            for img in range(2):
                t = xpool.tile([128, 2 * WP], F16, tag="xp2", name=f"xpf{img}")
                load_strip2(t, img, 0)
                xp_first.append(t)

            # per-sample dual-band matrices: separate tiles => separate
            # dependency tracking; later samples load lazily
            a_t = [
                apool.tile([128, N_DX, M_STRIP], F16, tag=f"a{s}",
                           name=f"a{s}")
                for s in range(BS)
            ]
            nc.sync.dma_start(out=a_t[0][:], in_=a_d[0])

            a_loaded = 0
            for pair in range(NIMG // 2):
                img_a, img_b = 2 * pair, 2 * pair + 1
                smp_a, smp_b = img_a // C, img_b // C
                for s_need in ((2 * pair + 2) // C, (2 * pair + 3) // C):
                    if s_need < BS and s_need > a_loaded:
                        nc.sync.dma_start(out=a_t[s_need][:], in_=a_d[s_need])
                        a_loaded = s_need

                # five double-strip units (rows 0..499) + one single overlap
                # strip at r0=462 storing rows 500..511
                for du in range(6):
                    if du < 5:
                        r0 = 100 * du
                        if pair == 0 and du == 0:
                            xa, xb = xp_first
                        else:
                            xa = xpool.tile([128, 2 * WP], F16, tag="xp2",
                                            name="xa")
                            load_strip2(xa, img_a, r0)
                            xb = xpool.tile([128, 2 * WP], F16, tag="xp2",
                                            name="xb")
                            load_strip2(xb, img_b, r0)
                        o_t = opool.tile([128, 2 * W], F32)
                        for sub in range(2):
                            base = sub * WP
                            acc = psum.tile([128, W], F32)
                            # all 8 streams use K=128 (j=7's upper band is
                            # zero weights) — a K=64 stream would switch the
                            # PE tiling mode and pay a drain twice per strip
                            for j in range(N_DX):
                                nc.tensor.matmul(
                                    acc[0:M_STRIP],
                                    a_t[smp_a][:, j, :],
                                    xa[:, base + 2 * j:base + 2 * j + W],
                                    start=(j == 0),
                                    stop=(j == N_DX - 1),
                                    tile_position=(0, 0),
                                )
                                nc.tensor.matmul(
                                    acc[64:64 + M_STRIP],
                                    a_t[smp_b][:, j, :],
                                    xb[:, base + 2 * j:base + 2 * j + W],
                                    start=(j == 0),
                                    stop=(j == N_DX - 1),
                                    tile_position=(0, 64),
                                )
                            nc.vector.tensor_copy(
                                out=o_t[:, sub * W:(sub + 1) * W],
                                in_=acc[:])
                        # one store per image covers both strips (100
                        # contiguous output rows; non-overlapping views)
                        dva = out_d[img_a, r0:r0 + 2 * M_STRIP, :].rearrange(
                            "(q p) c -> p q c", q=2)
                        dvb = out_d[img_b, r0:r0 + 2 * M_STRIP, :].rearrange(
                            "(q p) c -> p q c", q=2)
                        sva = o_t[0:M_STRIP, :].rearrange(
                            "p (q c) -> p q c", c=W)
                        svb = o_t[64:64 + M_STRIP, :].rearrange(
                            "p (q c) -> p q c", c=W)
                        nc.scalar.dma_start(out=dva, in_=sva)
                        nc.scalar.dma_start(out=dvb, in_=svb)
                    else:
                        r0 = R0S[-1]  # 462
                        lo = 10 * M_STRIP - r0  # store rows 500..511 only
                        xa = xpool.tile([128, WP], F16, tag="xp_t", name="xa1")
                        load_strip(xa, img_a, r0)
                        xb = xpool.tile([128, WP], F16, tag="xp_t", name="xb1")
                        load_strip(xb, img_b, r0)
                        acc = psum.tile([128, W], F32)
                        for j in range(N_DX):
                            nc.tensor.matmul(
                                acc[0:M_STRIP], a_t[smp_a][:, j, :],
                                xa[:, 2 * j:2 * j + W], start=(j == 0),
                                stop=(j == N_DX - 1), tile_position=(0, 0))
                            nc.tensor.matmul(
                                acc[64:64 + M_STRIP], a_t[smp_b][:, j, :],
                                xb[:, 2 * j:2 * j + W], start=(j == 0),
                                stop=(j == N_DX - 1), tile_position=(0, 64))
                        o_s = opool.tile([128, W], F32, tag="o1", name="o1")
                        nc.vector.tensor_copy(out=o_s[:], in_=acc[:])
                        nc.scalar.dma_start(
                            out=out_d[img_a, r0 + lo:r0 + M_STRIP, :],
                            in_=o_s[lo:M_STRIP])
                        nc.scalar.dma_start(
                            out=out_d[img_b, r0 + lo:r0 + M_STRIP, :],
                            in_=o_s[64 + lo:64 + M_STRIP])
    nc.compile()
    return nc


def prepare_in_maps(x: np.ndarray, kern: np.ndarray) -> list:
    # host-side reflection pad, cast to fp16 for half the DMA bytes
    xp = np.pad(x, ((0, 0), (0, 0), (P, P), (P, P)), mode="reflect")
    xp = np.ascontiguousarray(
        xp.reshape(B * C, HP, WP).astype(np.float16))

    # dual-band matrices: lower band = even taps, upper band = odd taps
    kern16 = kern.astype(np.float16)
    a_all = np.zeros((B, 128, N_DX, M_STRIP), dtype=np.float16)
    m_idx = np.arange(M_STRIP)
    for dy in range(L):
        a_all[:, m_idx + dy, :, m_idx] = kern16[:, dy, 0::2]
        a_all[:, K_GRP + m_idx + dy, :L // 2, m_idx] = kern16[:, dy, 1::2]

    return [
        {
            "xp": xp[c * NIMG:(c + 1) * NIMG],
            "a": a_all[c * BS:(c + 1) * BS],
        }
        for c in range(N_CORES)
    ]


def kernel(x: np.ndarray, kernel: np.ndarray) -> np.ndarray:
    global _program_cache
    x = np.asarray(x, dtype=np.float32)
    kern = np.asarray(kernel, dtype=np.float32)

    in_maps = prepare_in_maps(x, kern)
    if _program_cache is None:
        _program_cache = _build_program()
    nc = _program_cache

    res = run_bass_kernel_spmd(nc, in_maps, core_ids=list(range(N_CORES)))
    out = np.concatenate([r["out"] for r in res.results], axis=0)
    return out.reshape(B, C, H, W)
